# revision 18
# baseline (speedup 1.0000x reference)
"""Trainium2 Bass kernel for nn_HOR_16870631539538 (dense_transformer).

Module (per batch item b, C=64 channels, hw=4096 spatial):
  stage 1: p = x_low^T conv attention [hw,hw], softmax over axis n,
           e = p_sm @ v + x_low
  stage 2: t = conv_e(e) @ xl2_sp (64x64), softmax over c, out = x_mid @ t_sm

Sharding: 8 cores = 4 batch items x 2 halves of the softmax-column dim (m).
Downstream of e only G = e @ xl2_sp (+ bias row) is needed, which is linear
in the m-partial e, so the cross-core combine is a 16KB AllReduce of G.

v2 schedule (vs baseline):
  - K=64 matmuls throughout (measured: column rate is K-independent; the old
    2.2x was the PE p-state ramp).  No zero-pad memsets.
  - v_sp / xl2sp produced directly in [n, c] layout by using the channel-major
    input chunk as the matmul stationary (lhsT); bias via a ones-row (K=65).
  - d-denominators via gpsimd reduce over the bf16 slab (ACT runs pure exp).
  - software-pipelined m-loop: pT[i+1,0] is emitted before extras so ACT
    never bubbles at iteration boundaries; e-acc[i-1] interleaves between
    pT passes in 4+4 bursts; mm-pool allocations kept in pairs to preserve
    double-buffer parity.
  - e-accs split into A (m-chunks 0-7) and B (8-15): A is finalized and
    AllReduced mid-loop (overlapped); the tail only pays B + a warm AR.
  - x_low residual enters accs-A via 0.5*I matmuls (spread in iters 2-3);
    the Sum_n xl2 row via a reduce of xlat and one 65-deep matmul.
"""

import numpy as np

import concourse.bacc as bacc
import concourse.bass as bass
import concourse.mybir as mybir
import concourse.tile as tile
from concourse.bass_utils import run_bass_kernel_spmd

dt = mybir.dt
AF = mybir.ActivationFunctionType
ALU = mybir.AluOpType
AX = mybir.AxisListType

N_CORES = 8
C = 64
HW = 4096
MH = HW // 2           # per-core m-half (2048)
NCH = MH // 128        # 16 m-chunks of 128 rows
NB = HW // 512         # 8 n-blocks of 512

DT_IN = dt.float16     # inputs + conv weights + pT operands
DT_SLAB = dt.bfloat16  # exp slab + vs (range-safe for exp w/o max)
DT_OUT = dt.float16    # tsm / xmidT operands for the output matmul
f32 = dt.float32

USE_COLLECTIVE = True
DSUM_GPSIMD = True     # d-sums on DVE slab reduce (else ACT accum_out)
PRE_RAMP = 6           # dummy 512-col matmuls to spin up the PE p-state
TRUNC = 99            # debug: 1=convs 2=+mloop 3=+G 99=full

_CACHE = {}


def build():
    nc = bacc.Bacc("TRN2", target_bir_lowering=False, debug=False,
                   num_devices=N_CORES)

    def din(name, shape, dtype):
        return nc.dram_tensor(name, shape, dtype, kind="ExternalInput").ap()

    xin = din("xin", [C, HW], DT_IN)      # x[b] channel-major
    xlat = din("xlat", [C, HW], DT_IN)    # x_latter[b]
    # blob16 [65, 7C]: wlT | whT | wmT | wv_ext | wlat_ext | halfid | wlat_half
    blob16 = din("blob16", [65, 7 * C], DT_IN)
    # blob32 [128, 195]: weT+be (c0:64) | I at rows 0:64 AND 64:128 (c64:128)
    #                    | bl | bh | bm columns | 0.5*[wlatT; blat] (c131:195)
    blob32 = din("blob32", [128, 2 * C + 3 + C], f32)
    outp = nc.dram_tensor("outp", [C, MH], f32, kind="ExternalOutput").ap()

    with tile.TileContext(nc) as tc:
        _body(nc, tc, locals())
    nc.compile()
    return nc


def _body(nc, tc, io):
    ts = bass.ts

    const = tc.alloc_tile_pool(name="const", bufs=1)
    big = tc.alloc_tile_pool(name="big", bufs=1)
    slabp = tc.alloc_tile_pool(name="slabp", bufs=2)
    mm = tc.alloc_tile_pool(name="mm", bufs=2, space="PSUM")
    acc = tc.alloc_tile_pool(name="acc", bufs=1, space="PSUM")
    dram = tc.alloc_tile_pool(name="dram", bufs=1, space="DRAM")

    # ---- constants ----
    blob16 = const.tile([65, 7 * C], DT_IN, tag="blob16")
    blob32 = const.tile([128, 2 * C + 3 + C], f32, tag="blob32")
    wlT = blob16[0:C, 0 * C:1 * C]
    whT = blob16[0:C, 1 * C:2 * C]
    wmT = blob16[0:C, 2 * C:3 * C]
    wv_e = blob16[0:65, 3 * C:4 * C]       # [wvT; bv row]
    wlat_e = blob16[0:65, 4 * C:5 * C]     # [wlatT; blat row]
    halfid = blob16[0:C, 5 * C:6 * C]      # 0.5*I
    weT_e = blob32[0:65, 0:C]              # [weT; be row]
    idf32 = blob32[0:C, C:2 * C]
    idhi = blob32[64:128, C:2 * C]
    bl = blob32[0:C, 2 * C + 0:2 * C + 1]
    bh = blob32[0:C, 2 * C + 1:2 * C + 2]
    bm = blob32[0:C, 2 * C + 2:2 * C + 3]
    wlat_h = blob32[0:65, 2 * C + 3:2 * C + 3 + C]  # 0.5*[wlatT; blat row]

    # ---- SBUF working set ----
    xin = big.tile([65, HW], DT_IN, tag="xin")      # row 64 = ones
    xlat = big.tile([65, HW], DT_IN, tag="xlat")    # row 64 = ones
    xlowT = big.tile([C, HW], DT_IN, tag="xlowT")
    xl_hi = big.tile([C, MH], DT_IN, tag="xl_hi")
    xmidT = big.tile([C, MH], DT_OUT, tag="xmidT")
    v_sp = big.tile([128, NCH * C], f32, tag="v_sp")
    xl2sp = big.tile([128, 32 * C], f32, tag="xl2sp")
    gs_acc = big.tile([65, C], f32, tag="gs_acc")
    sacc = big.tile([C, 4], f32, tag="sacc")
    xsum65 = big.tile([65, 1], f32, tag="xsum65")
    junk = big.tile([C, 576], DT_IN, tag="junk")
    trash = None
    if DSUM_GPSIMD:
        trash = big.tile([128, HW], DT_SLAB, tag="trash", name="trash")

    # ---- prologue DMAs.  SP carries xin (gates the iter-0 pT passes) in
    # consumption order; ACT takes the xlat tail (loose deadlines).  gpsimd
    # does the small memsets. ----
    nc.gpsimd.memset(junk[:], 0.125)
    nc.gpsimd.memset(xin[64:65, :], 1.0)
    nc.gpsimd.memset(xlat[64:65, :], 1.0)
    nc.gpsimd.memset(gs_acc[:], 0.0)
    nc.gpsimd.memset(xsum65[64:65, :], float(HW))

    nc.sync.dma_start(blob16[:], io["blob16"])
    nc.sync.dma_start(blob32[:], io["blob32"])
    nc.sync.dma_start(xin[0:32, 0:1024], io["xin"][0:32, 0:1024])
    nc.sync.dma_start(xin[32:64, 0:1024], io["xin"][32:64, 0:1024])
    nc.sync.dma_start(xlat[0:64, 0:256], io["xlat"][0:64, 0:256])
    nc.sync.dma_start(xin[0:32, 1024:2048], io["xin"][0:32, 1024:2048])
    nc.sync.dma_start(xin[32:64, 1024:2048], io["xin"][32:64, 1024:2048])
    nc.sync.dma_start(xin[0:32, 2048:3072], io["xin"][0:32, 2048:3072])
    nc.sync.dma_start(xin[32:64, 2048:3072], io["xin"][32:64, 2048:3072])
    nc.sync.dma_start(xin[0:32, 3072:4096], io["xin"][0:32, 3072:4096])
    nc.sync.dma_start(xin[32:64, 3072:4096], io["xin"][32:64, 3072:4096])

    nc.scalar.dma_start(xlat[0:64, 256:1024], io["xlat"][0:64, 256:1024])
    nc.scalar.dma_start(xlat[0:32, 1024:2560], io["xlat"][0:32, 1024:2560])
    nc.scalar.dma_start(xlat[32:64, 1024:2560], io["xlat"][32:64, 1024:2560])
    nc.scalar.dma_start(xlat[0:32, 2560:4096], io["xlat"][0:32, 2560:4096])
    nc.scalar.dma_start(xlat[32:64, 2560:4096], io["xlat"][32:64, 2560:4096])

    # warmup collective: throwaway pair AllReduce so the CC mesh is warm.
    if USE_COLLECTIVE:
        warm_in = dram.tile([1, 16], f32, tag="warm_in")
        warm_out = dram.tile([1, 16], f32, tag="warm_out")
        nc.sync.dma_start(warm_in[:], blob32[0:1, 0:16])
        nc.gpsimd.collective_compute(
            "AllReduce", ALU.add,
            ins=[warm_in.opt()], outs=[warm_out.opt()],
            replica_groups=[[0, 1], [2, 3], [4, 5], [6, 7]],
        )

    # ---- PE p-state pre-ramp on junk data (no consumers) ----
    for _ in range(PRE_RAMP):
        pt = mm.tile([128, 1024], f32, tag="mmt")
        nc.tensor.matmul(pt[0:C, 0:512], junk[:, 0:C], junk[:, C:C + 512],
                         start=True, stop=True)

    # ---- conv helper: dst[c, cols] = W^T-contract(src) + bias, DVE evict ----
    def conv(dst, wT, src, bias, j0, width):
        pt = mm.tile([128, 1024], f32, tag="mmt")
        for k in range(0, width, 512):
            w = min(512, width - k)
            nc.tensor.matmul(pt[0:C, k:k + w], wT, src[0:C, j0 + k:j0 + k + w],
                             start=True, stop=True)
        nc.vector.tensor_scalar(dst[0:C, j0:j0 + width], pt[0:C, 0:width],
                                bias, None, ALU.add)

    # direct-transpose conv group: 8 tiles [128, 64] of dst (layout [n, c])
    # from ext-input chunks as lhsT; bias via the ones row (K=65).
    def sp_group(dst, src_ext, w_ext, g):
        pt = mm.tile([128, 1024], f32, tag="mmt")
        for q in range(8):
            i = g + q
            nc.tensor.matmul(pt[:, q * 64:(q + 1) * 64],
                             src_ext[0:65, ts(i, 128)], w_ext,
                             start=True, stop=True)
        nc.vector.tensor_copy(dst[:, g * 64:(g + 8) * 64], pt[:, 0:512])

    # ---- e^T accumulators: 4 psum tiles [128, 512]; partition half p holds
    # n-block 2k+p.  Group A = m-chunks 0-7 (+ residual), group B = 8-15. ----
    def alloc_accs():
        return [acc.tile([128, 512], f32, tag=f"acc{k}", name=f"acc{k}")
                for k in range(4)]

    st = {"accs": alloc_accs(), "eh": None, "esp": None}

    def eacc_burst(i, vs, slab, ks):
        first = i in (0, 8)
        last = i in (7, 15)
        for k in ks:
            for p in range(2):
                blk = 2 * k + p
                nc.tensor.matmul(st["accs"][k][p * 64:(p + 1) * 64, :], vs,
                                 slab[:, ts(blk, 512)], start=first,
                                 stop=last, skip_group_check=True)

    def resid_pair(ks):
        # x_low residual into accs-A: 0.5*I matmuls (0.5: the pair AllReduce
        # sums two copies).  Order-free within the accumulation group.
        for k in ks:
            for p in range(2):
                blk = 2 * k + p
                nc.tensor.matmul(st["accs"][k][p * 64:(p + 1) * 64, :], halfid,
                                 xlowT[:, ts(blk, 512)], start=False,
                                 stop=False, skip_group_check=True)

    def evict_accs():
        # DVE evict, then partition-move the high halves 64:128 -> eh2 rows
        # 0:64 via SBUF DMAs (PE transposes cannot read partitions 64-127)
        eh = big.tile([128, 4 * 512], f32, tag="eh", name="eh")
        eh2 = big.tile([64, 4 * 512], f32, tag="eh2", name="eh2")
        for k in range(4):
            nc.vector.tensor_copy(eh[:, ts(k, 512)], st["accs"][k][:])
            nc.sync.dma_start(eh2[0:32, ts(k, 512)], eh[64:96, ts(k, 512)])
            nc.sync.dma_start(eh2[32:64, ts(k, 512)], eh[96:128, ts(k, 512)])
        st["eh"] = eh
        st["eh2"] = eh2

    def transpose_pair(g2, fresh):
        # 8 transposes -> one mm tile, one DVE copy into e_sp cols g2*512..
        if fresh:
            st["esp"] = big.tile([128, 32 * C], f32, tag="esp", name="esp")
        esp, eh = st["esp"], st["eh"]
        pt = mm.tile([128, 1024], f32, tag="mmt")
        for q in range(8):
            t_idx = g2 * 8 + q
            blk, sl = t_idx // 4, t_idx % 4
            k, p = blk // 2, blk % 2
            lo = 64 * p
            eh_src = eh if p == 0 else st["eh2"]
            src = eh_src[0:64, k * 512 + sl * 128:k * 512 + (sl + 1) * 128]
            nc.tensor.transpose(pt[:, q * 64:(q + 1) * 64], src, idf32)
            _ = lo
        nc.vector.tensor_copy(esp[:, g2 * 512:(g2 + 1) * 512], pt[:, 0:512])

    def g_burst(g, with_srow, to_sbuf):
        # 8 G matmuls accumulated in one psum tile, then DVE-add into gs_acc
        esp = st["esp"]
        pt = mm.tile([128, 1024], f32, tag="mmt")
        G = pt[0:64, 0:64]
        for q in range(8):
            t_idx = g * 8 + q
            nc.tensor.matmul(G, esp[:, t_idx * 64:(t_idx + 1) * 64],
                             xl2sp[:, ts(t_idx, C)],
                             start=(q == 0), stop=(q == 7),
                             skip_group_check=True)
        if with_srow:
            # row 64: 0.5*(W_lat xsum + 4096 b_lat) = 0.5*Sum_n xl2 row
            nc.tensor.matmul(pt[64:65, 0:64], xsum65, wlat_h,
                             start=True, stop=True, skip_group_check=True)
        if to_sbuf:
            rows = 65 if with_srow else 64
            nc.vector.tensor_tensor(gs_acc[0:rows, :], gs_acc[0:rows, :],
                                    pt[0:rows, 0:64], ALU.add)
        return pt

    def finish(src_ap):
        osb = big.tile([C, MH], f32, tag="osb")
        nc.gpsimd.memset(osb[:], 0.0)
        nc.vector.tensor_copy(osb[0:src_ap.shape[0], 0:src_ap.shape[1]],
                              src_ap)
        nc.sync.dma_start(io["outp"], osb[:])
        for p in (dram, acc, mm, slabp, big, const):
            p.release()

    # ---- prologue compute: first conv chunks + iter-0 pass 0 ----
    conv(xlowT, wlT, xin, bl, 0, 1024)
    conv(xlowT, wlT, xin, bl, 1024, 1024)
    conv(xl_hi, whT, xlat, bh, 0, 256)

    if TRUNC == 1:
        conv(xlowT, wlT, xin, bl, 2048, 1024)
        conv(xlowT, wlT, xin, bl, 3072, 1024)
        return finish(xlowT[0:C, 0:MH])

    def pT_pass(i, j):
        pt = mm.tile([128, 1024], f32, tag="mmt")
        for k in range(2):
            nc.tensor.matmul(pt[:, k * 512:(k + 1) * 512],
                             xl_hi[:, ts(i, 128)],
                             xlowT[:, j * 1024 + k * 512:
                                   j * 1024 + (k + 1) * 512],
                             start=True, stop=True)
        return pt

    # PE extras per iteration: list of closures, each allocating exactly one
    # mm tile (keep counts EVEN to preserve pT double-buffer parity).
    pe_extras = {
        0: [lambda: conv(xl_hi, whT, xlat, bh, 256, 512),
            lambda: conv(xl_hi, whT, xlat, bh, 768, 512),
            lambda: sp_group(v_sp, xin, wv_e, 0),
            lambda: sp_group(v_sp, xin, wv_e, 8)],
        1: [lambda: conv(xl_hi, whT, xlat, bh, 1280, 512),
            lambda: conv(xl_hi, whT, xlat, bh, 1792, 256)],
        2: [lambda: sp_group(xl2sp, xlat, wlat_e, 0),
            lambda: sp_group(xl2sp, xlat, wlat_e, 8)],
        3: [lambda: sp_group(xl2sp, xlat, wlat_e, 16),
            lambda: sp_group(xl2sp, xlat, wlat_e, 24)],
        4: [lambda: conv(xmidT, wmT, xin, bm, 0, 1024),
            lambda: conv(xmidT, wmT, xin, bm, 1024, 1024)],
        9: [lambda: transpose_pair(0, True),
            lambda: transpose_pair(1, False)],
        10: [lambda: transpose_pair(2, False),
             lambda: transpose_pair(3, False)],
        11: [lambda: g_burst(0, True, True),
             lambda: g_burst(1, False, True)],
        12: [lambda: g_burst(2, False, True),
             lambda: g_burst(3, False, True)],
    }
    if TRUNC < 3:
        for k in (9, 10, 11, 12):
            pe_extras.pop(k)

    # extras emitted between pT passes of an iteration: (i, after_pass_j)
    pe_mid_extras = {
        (0, 1): [lambda: conv(xlowT, wlT, xin, bl, 2048, 1024)],
        (0, 2): [lambda: conv(xlowT, wlT, xin, bl, 3072, 1024)],
    }
    # PE non-alloc extras (accumulate into accs; no mm tiles)
    pe_acc_extras = {
        2: [lambda: resid_pair((0, 1))],
        3: [lambda: resid_pair((2, 3))],
    }
    # DVE extras (beyond rec/vs and the copies the closures above make)
    dve_extras = {
        3: [lambda: nc.vector.reduce_sum(sacc[:, 0:1], xlat[0:64, 0:1024],
                                         axis=AX.X)],
        4: [lambda: nc.vector.reduce_sum(sacc[:, 1:2], xlat[0:64, 1024:2048],
                                         axis=AX.X)],
        5: [lambda: nc.vector.reduce_sum(sacc[:, 2:3], xlat[0:64, 2048:3072],
                                         axis=AX.X),
            lambda: nc.vector.reduce_sum(sacc[:, 3:4], xlat[0:64, 3072:4096],
                                         axis=AX.X)],
        6: [lambda: nc.vector.reduce_sum(xsum65[0:64, :], sacc[:], axis=AX.X)],
        8: [lambda: evict_accs()],
    }

    # ---- m-loop ----
    prev = None  # (vs, slab) of iteration i-1
    for i in range(NCH):
        if i == 9:
            # fresh accumulators for group B (after the A eviction)
            st["accs"] = alloc_accs()

        slab = slabp.tile([128, HW], DT_SLAB, tag="slab")
        dacc = None if DSUM_GPSIMD else slabp.tile([128, 4], f32, tag="dacc")

        def do_exp(j, pt):
            if dacc is None:
                nc.scalar.activation(slab[:, j * 1024:(j + 1) * 1024],
                                     pt[:], AF.Exp)
            else:
                nc.scalar.activation(slab[:, j * 1024:(j + 1) * 1024],
                                     pt[:], AF.Exp,
                                     accum_out=dacc[:, j:j + 1])

        pt0 = pT_pass(0, 0) if i == 0 else st.pop("pt_next")
        do_exp(0, pt0)
        for x in pe_mid_extras.get((i, 0), ()):
            x()
        pt1 = pT_pass(i, 1)
        do_exp(1, pt1)
        if prev is not None:
            eacc_burst(i - 1, prev[0], prev[1], (0, 1))
        for x in pe_mid_extras.get((i, 1), ()):
            x()
        pt2 = pT_pass(i, 2)
        do_exp(2, pt2)
        if prev is not None:
            eacc_burst(i - 1, prev[0], prev[1], (2, 3))
        for x in pe_mid_extras.get((i, 2), ()):
            x()
        pt3 = pT_pass(i, 3)
        do_exp(3, pt3)
        if i + 1 < NCH:
            st["pt_next"] = pT_pass(i + 1, 0)

        # d, 1/d, vs on DVE (emitted before the extras' DVE copies so the
        # e-acc chain of the next iteration is never queued behind them)
        dsum = slabp.tile([128, 1], f32, tag="dsum")
        if DSUM_GPSIMD:
            # free-axis sum of the bf16 slab on DVE (keeps ACT pure-exp)
            nc.vector.reduce_sum(dsum[:], slab[:], axis=AX.X)
        else:
            nc.vector.reduce_sum(dsum[:], dacc[:], axis=AX.X)
        rec = slabp.tile([128, 1], f32, tag="rec")
        nc.vector.reciprocal(rec[:], dsum[:])
        vs = slabp.tile([128, C], DT_SLAB, tag="vs")
        nc.vector.tensor_scalar(vs[:], v_sp[:, ts(i, C)], rec[:], None,
                                ALU.mult)

        for x in pe_acc_extras.get(i, ()):
            x()
        for x in pe_extras.get(i, ()):
            x()
        for x in dve_extras.get(i, ()):
            x()
        prev = (vs, slab)

        if i == 12 and USE_COLLECTIVE:
            # mid-loop AllReduce of the A-half payload (overlapped)
            g_inA = dram.tile([65, C], f32, tag="g_inA")
            g_outA = dram.tile([65, C], f32, tag="g_outA")
            nc.sync.dma_start(g_inA[0:33, :], gs_acc[0:33, :])
            nc.sync.dma_start(g_inA[33:65, :], gs_acc[33:65, :])
            nc.gpsimd.collective_compute(
                "AllReduce", ALU.add,
                ins=[g_inA.opt()], outs=[g_outA.opt()],
                replica_groups=[[0, 1], [2, 3], [4, 5], [6, 7]],
            )
            gs_redA = big.tile([65, C], f32, tag="gs_redA")
            nc.sync.dma_start(gs_redA[0:33, :], g_outA[0:33, :])
            nc.sync.dma_start(gs_redA[33:65, :], g_outA[33:65, :])
            st["gs_redA"] = gs_redA

    # ---- tail: close B, AllReduce it, stage 2, output ----
    eacc_burst(15, prev[0], prev[1], (0, 1, 2, 3))
    evict_accs()
    if TRUNC == 2:
        return finish(st["eh"][0:64, 0:MH])
    transpose_pair(0, True)
    transpose_pair(1, False)
    transpose_pair(2, False)
    transpose_pair(3, False)

    # single accumulated G_B psum (32 matmuls), then stage to SBUF via DVE
    esp = st["esp"]
    gpsB = mm.tile([128, 1024], f32, tag="mmt")
    GB = gpsB[0:64, 0:64]
    for t_idx in range(32):
        nc.tensor.matmul(GB, esp[:, t_idx * 64:(t_idx + 1) * 64],
                         xl2sp[:, ts(t_idx, C)],
                         start=(t_idx == 0), stop=(t_idx == 31),
                         skip_group_check=True)
    gs_stB = big.tile([C, C], f32, tag="gs_stB")
    nc.vector.tensor_copy(gs_stB[:], GB)
    if TRUNC == 3:
        return finish(gs_stB[:])

    if USE_COLLECTIVE:
        g_inB = dram.tile([C, C], f32, tag="g_inB")
        g_outB = dram.tile([C, C], f32, tag="g_outB")
        nc.sync.dma_start(g_inB[0:32, :], gs_stB[0:32, :])
        nc.sync.dma_start(g_inB[32:64, :], gs_stB[32:64, :])
        nc.gpsimd.collective_compute(
            "AllReduce", ALU.add,
            ins=[g_inB.opt()], outs=[g_outB.opt()],
            replica_groups=[[0, 1], [2, 3], [4, 5], [6, 7]],
        )
        gs_redB = big.tile([C, C], f32, tag="gs_redB")
        nc.sync.dma_start(gs_redB[0:32, :], g_outB[0:32, :])
        nc.sync.dma_start(gs_redB[32:64, :], g_outB[32:64, :])
        gs_redA = st["gs_redA"]
    else:
        gs_redA = gs_acc
        gs_redB = gs_stB

    # tT[d, c] accumulated from the A and B reduced payloads
    tps = mm.tile([128, 1024], f32, tag="mmt")
    tT = tps[0:64, 0:64]
    nc.tensor.matmul(tT, gs_redA[0:65, :], weT_e[0:65, :],
                     start=True, stop=False, skip_group_check=True)
    nc.tensor.matmul(tT, gs_redB[0:64, :], weT_e[0:64, :],
                     start=False, stop=True, skip_group_check=True)

    # softmax over c (free axis of tT): exp with max-subtraction
    tmax = big.tile([C, 1], f32, tag="tmax")
    nc.vector.reduce_max(tmax[:], tT, axis=AX.X)
    nmax = big.tile([C, 1], f32, tag="nmax")
    nc.vector.tensor_scalar(nmax[:], tmax[:], -1.0, None, ALU.mult)
    texp = big.tile([C, C], f32, tag="texp")
    tsum = big.tile([C, 1], f32, tag="tsum")
    nc.scalar.activation(texp[:], tT, AF.Exp, bias=nmax[:], accum_out=tsum[:])
    trec = big.tile([C, 1], f32, tag="trec")
    nc.vector.reciprocal(trec[:], tsum[:])

    # transpose unnormalized texp back -> [c, d] fp16; 1/tsum[d] folds into
    # the per-partition scale of the output evictions (out rows are d).
    tb = mm.tile([128, 1024], f32, tag="mmt")
    nc.tensor.transpose(tb[0:64, 0:64], texp[:], idf32)
    tsm = big.tile([C, C], DT_OUT, tag="tsm")
    nc.vector.tensor_copy(tsm[:], tb[0:64, 0:64])

    # out^T[d, n-half] = tsm-contract @ xmidT (K=64)
    osb = big.tile([C, MH], f32, tag="osb")
    for k in range(4):
        op = mm.tile([128, 1024], f32, tag="mmt")
        nc.tensor.matmul(op[0:C, 0:512], tsm[:], xmidT[:, ts(k, 512)],
                         start=True, stop=True)
        if k % 2 == 0:
            nc.vector.tensor_scalar(osb[:, ts(k, 512)], op[0:C, 0:512],
                                    trec[:], None, ALU.mult)
        else:
            nc.scalar.activation(osb[:, ts(k, 512)], op[0:C, 0:512],
                                 AF.Copy, scale=trec[:])
        nc.sync.dma_start(io["outp"][0:C, ts(k, 512)], osb[:, ts(k, 512)])

    for p in (dram, acc, mm, slabp, big, const):
        p.release()


def _prep_inputs(x_latter, x, W, b):
    """Build the 8 per-core input maps from full inputs."""
    B = x_latter.shape[0]
    xr = x.reshape(B, C, HW).astype(np.float16)
    xlr = x_latter.reshape(B, C, HW).astype(np.float16)
    wT = {k: np.ascontiguousarray(W[k].T) for k in W}

    blob16 = np.zeros((65, 7 * C), np.float16)
    blob16[0:C, 0 * C:1 * C] = wT["low"].astype(np.float16)
    blob16[0:C, 1 * C:2 * C] = wT["high"].astype(np.float16)
    blob16[0:C, 2 * C:3 * C] = wT["mid"].astype(np.float16)
    blob16[0:C, 3 * C:4 * C] = wT["value"].astype(np.float16)
    blob16[64, 3 * C:4 * C] = b["value"].reshape(C)
    blob16[0:C, 4 * C:5 * C] = wT["latter"].astype(np.float16)
    blob16[64, 4 * C:5 * C] = b["latter"].reshape(C)
    blob16[0:C, 5 * C:6 * C] = (0.5 * np.eye(C)).astype(np.float16)

    blob32 = np.zeros((128, 2 * C + 3 + C), np.float32)
    blob32[0:C, 0:C] = wT["e_conv"]
    blob32[64, 0:C] = b["e_conv"].reshape(C)
    blob32[0:C, C:2 * C] = np.eye(C, dtype=np.float32)
    blob32[C:2 * C, C:2 * C] = np.eye(C, dtype=np.float32)
    blob32[0:C, 2 * C + 0] = b["low"].reshape(C)
    blob32[0:C, 2 * C + 1] = b["high"].reshape(C)
    blob32[0:C, 2 * C + 2] = b["mid"].reshape(C)
    blob32[0:C, 2 * C + 3:2 * C + 3 + C] = 0.5 * wT["latter"]
    blob32[64, 2 * C + 3:2 * C + 3 + C] = 0.5 * b["latter"].reshape(C)

    maps = []
    for core in range(N_CORES):
        bi, h = core // 2, core % 2
        # roll columns so this core's own m-half sits at columns [0, MH)
        xin_c = np.roll(xr[bi], -h * MH, axis=1) if h else xr[bi]
        xlat_c = np.roll(xlr[bi], -h * MH, axis=1) if h else xlr[bi]
        maps.append({
            "xin": np.ascontiguousarray(xin_c),
            "xlat": np.ascontiguousarray(xlat_c),
            "blob16": blob16,
            "blob32": blob32,
        })
    return maps


def run(inputs, trace=False, trace_cores=None):
    if "nc" not in _CACHE:
        _CACHE["nc"] = build()
    nc = _CACHE["nc"]

    names = ["high", "low", "value", "e_conv", "mid", "latter"]
    W = {n: np.asarray(inputs[f"W_{n}"], dtype=np.float32) for n in names}
    b = {n: np.asarray(inputs[f"b_{n}"], dtype=np.float32).reshape(C, 1)
         for n in names}
    x = np.asarray(inputs["x"], dtype=np.float32)
    x_latter = np.asarray(inputs["x_latter"], dtype=np.float32)
    maps = _prep_inputs(x_latter, x, W, b)

    kw = {}
    if trace:
        kw = dict(trace=True,
                  trace_cores=trace_cores or list(range(N_CORES)))
    res = run_bass_kernel_spmd(nc, maps, core_ids=list(range(N_CORES)), **kw)

    B = x_latter.shape[0]
    out = np.empty((B, C, HW), dtype=np.float32)
    for core in range(N_CORES):
        bi, h = core // 2, core % 2
        out[bi][:, h * MH:(h + 1) * MH] = res.results[core]["outp"]
    H = int(np.sqrt(HW))
    return out.reshape(B, C, H, H), res


def kernel(**inputs):
    out, _ = run(inputs, trace=False)
    return out
